# revision 29
# baseline (speedup 1.0000x reference)
"""Bass/Trainium2 kernel for nn_BipolarMorphological2D.

Math: reference computes, per branch,
    y = exp(max_p(log(max(patch, 0.1)) + k[p, o]))
      = max_p(max(patch, 0.1) * exp(k)[p, o])
a tropical (max-times) matmul with positive operands.  We approximate the
max-reduction with a high-order p-norm computed on the TensorEngine:
    max_p(z_p) ~= (sum_p z_p^beta)^(1/beta),  beta = 112
Since (u * e)^beta = u^beta * e^beta, each S = sum_p u^beta E^beta is a plain
matmul of patch powers against exp(beta*k) -- PE work instead of DVE work.
The p-norm overestimates when several z_p tie near the max; to suppress that,
P=288 is split into 5 chunks (pairs of 3x3 tap positions), each chunk gets its
own p-norm, and the DVE takes the exact max across chunk sums (monotone, so the
max can be taken on raw S values before the 1/beta root).  Measured rel-l2 vs
the exact reference on hardware: 9.96e-3 (threshold 2e-2).

Scaling: alpha=0.28 keeps S = sum (alpha*u*E)^112 in [e^-79, e^44]; one ACT
Sqrt pass then fits it into Ln's hardware window (2^-64, 2^64).

Per core (one image, batch-parallel across 8 cores):
  DVE:  clamp+scale u = (x max 0.1)*alpha; u^112 = u^16*u^32*u^64 products;
        20 chunk-max folds; final combine.  ACT: squaring chain to u^64 (fp32
        internal), exp(112 k), sqrt+ln+exp root.  PE: 36 matmuls, K=32, N=450
        (h,w)-strided AP covering only valid output pixels, ij-pairs
        accumulated in PSUM.  The squaring chain is split into two halo'd
        column halves so matmuls overlap it; sign 0's root overlaps sign 1's
        matmuls; the e2->e1 partition move rides a small Lhi DMA overlapped
        with the exp stage.
"""

import numpy as np

B, C, H, W, O = 8, 32, 32, 32, 64
FH, FW = 3, 3
HO, WO = H - FH + 1, W - FW + 1  # 30, 30
NG = 2 * O                       # 128 output groups: (kernel e1/e2) x (o)
NCORES = 8

BETA = 112
ALPHA = 0.28
SHIFT = 0.1
NSQ = 6                          # squarings to u^64; u^112 = u^64*u^32*u^16
HHALF = HO // 2                  # 15 rows per matmul half
NFREE = HHALF * WO               # 450 columns per matmul (one PSUM bank)

# chunk structure: 5 chunks = pairs of ij taps (last chunk is ij=8 alone)
CHUNKS = [(0, 1), (2, 3), (4, 5), (6, 7), (8,)]

_CACHE = {}


def _build_program(reps=1, outer=1, dump=None):
    key = ("nc", reps, outer, dump)
    if key in _CACHE:
        return _CACHE[key]

    import concourse.mybir as mybir
    import concourse.tile as tile
    from concourse import bacc

    f32 = mybir.dt.float32
    bf16 = mybir.dt.bfloat16
    Alu = mybir.AluOpType
    Act = mybir.ActivationFunctionType

    nc = bacc.Bacc()

    xp = nc.dram_tensor("xp", [2 * C, H * W], f32, kind="ExternalInput")
    kT = nc.dram_tensor("kT", [C, FH * FW * NG], f32, kind="ExternalInput")
    biasb = nc.dram_tensor("biasb", [O, 1], f32, kind="ExternalInput")
    y = nc.dram_tensor("y", [O, HO * WO], f32, kind="ExternalOutput")

    with tile.TileContext(nc) as tc:
        with tc.tile_pool(name="const", bufs=1) as cpool, \
             tc.tile_pool(name="psum", bufs=8, space="PSUM") as ppool:

            HWP = 17 * W
            KT = cpool.tile([C, FH * FW, NG], f32)
            nc.sync.dma_start(KT[:], kT[:].rearrange("c (a g) -> c a g", g=NG))
            XP = [cpool.tile([2 * C, HWP], f32, name=f'x{p}') for p in range(2)]
            nc.sync.dma_start(XP[0][:], xp[:, 0:HWP])
            nc.sync.dma_start(XP[1][:], xp[:, HHALF * W:])
            Bt = cpool.tile([O, 1], f32)
            nc.sync.dma_start(Bt[:], biasb[:])
            NLA = cpool.tile([NG, 1], f32)
            nc.gpsimd.memset(NLA[:], -float(np.log(ALPHA)))

            # exp(beta * k) in bf16 on rows 0:32, then DMA-duplicate to rows
            # 32:64 (matmul lhsT must share base partition with each sign's rhs)
            ET = cpool.tile([2 * C, FH * FW, NG], bf16)
            nc.scalar.activation(ET[0:C], KT[:], Act.Exp, scale=float(BETA))
            nc.sync.dma_start(ET[C:2 * C], ET[0:C])

            UP = [cpool.tile([2 * C, HWP], f32, name=f'u{p}') for p in range(2)]
            Upow = [cpool.tile([2 * C, HWP], f32, name=f'upow{n}')
                    for n in range(2 * NSQ)]
            M1P = [cpool.tile([2 * C, HWP], f32, name=f'm1{p}') for p in range(2)]
            RP = [cpool.tile([2 * C, HWP], bf16, name=f'r{p}') for p in range(2)]
            SM = cpool.tile([NG, 2, 2, NFREE], bf16)   # (sign, half) chunk-max
            Q1 = cpool.tile([NG, 2, HO * WO], f32)
            Q2 = cpool.tile([NG, 2, HO * WO], f32)
            L = cpool.tile([NG, 2, HO * WO], f32)
            Y = cpool.tile([NG, 2, HO * WO], f32)
            Lhi = cpool.tile([O, 2, HO * WO], f32)
            Yhi = cpool.tile([O, 2, HO * WO], f32)
            D1 = cpool.tile([O, HO * WO], f32)
            D2 = cpool.tile([O, HO * WO], f32)
            Rt = cpool.tile([O, HO * WO], f32)

            for _ in range(reps):
                # u = (x max 0.1) * alpha, both signs packed on 64
                # partitions; the squaring chain runs per column part (rows
                # 0:17 and 15:32 of the image, 2-row halo) so part A's
                # matmuls can start while part B is still powering.
                # part 0's squaring chain on ACT, part 1's on DVE -- the two
                # engines power their halves concurrently
                for p in range(2):
                    nc.vector.tensor_scalar(
                        out=UP[p][:], in0=XP[p][:],
                        scalar1=SHIFT, scalar2=ALPHA,
                        op0=Alu.max, op1=Alu.mult)
                for p in range(2):
                    prev = UP[p]
                    for sq in range(NSQ):
                        o = Upow[p * NSQ + sq]
                        if p == 0:
                            nc.scalar.activation(o[:], prev[:], Act.Square)
                        else:
                            nc.vector.tensor_tensor(
                                out=o[:], in0=prev[:], in1=prev[:],
                                op=Alu.mult)
                        prev = o
                    nc.vector.tensor_tensor(
                        out=M1P[p][:], in0=Upow[p * NSQ + 3][:],
                        in1=Upow[p * NSQ + 4][:], op=Alu.mult)
                    nc.vector.tensor_tensor(
                        out=RP[p][:], in0=M1P[p][:], in1=Upow[p * NSQ + 5][:],
                        op=Alu.mult)

                Rvp = [RP[p][:].rearrange("p (h w) -> p h w", w=W)
                       for p in range(2)]

                # tropical matmuls + chunk-max folds, sign-outer so sign 0's
                # root phase overlaps sign 1's matmul phase.  Within a chunk
                # the two h-halves share one LDWEIGHTS per tap.
                for sign in range(2):
                    for ci, chunk in enumerate(CHUNKS):
                        pss = []
                        for half in range(2):
                            ps = ppool.tile([NG, NFREE], f32, name="ps")
                            pss.append(ps)
                        for m, ij in enumerate(chunk):
                            i, j = divmod(ij, FW)
                            for half in range(2):
                                rhs = Rvp[half][C * sign:C * (sign + 1),
                                               i:i + HHALF, j:j + WO]
                                nc.tensor.matmul(
                                    pss[half][:],
                                    ET[C * sign:C * (sign + 1), ij, :], rhs,
                                    start=(m == 0), stop=(m == len(chunk) - 1))
                        for half in range(2):
                            dstv = SM[:, sign, half, :]
                            if ci == 0:
                                nc.vector.tensor_scalar(
                                    out=dstv, in0=pss[half][:], scalar1=0.0,
                                    scalar2=None, op0=Alu.max)
                            else:
                                nc.vector.tensor_tensor(
                                    out=dstv, in0=dstv, in1=pss[half][:],
                                    op=Alu.max)
                    # one Sqrt pass compresses S into Ln's (2^-64, 2^64)
                    # window (S in [e^-79, e^44] -> [e^-40, e^22]); sign 0's
                    # runs under sign 1's matmul phase
                    nc.scalar.activation(
                        Q1[:, sign], SM[:, sign].rearrange("p a f -> p (a f)"),
                        Act.Sqrt)
                    nc.scalar.activation(L[:, sign], Q1[:, sign], Act.Ln)
                    nc.sync.dma_start(Lhi[:, sign], L[O:NG, sign])

                # root tail: jumbo Ln across signs (keeps the scheduler from
                # thrashing ACT table sets), early Lhi DMA, then per-block exps
                for sign in range(2):
                    nc.scalar.activation(
                        Y[0:O, sign], L[0:O, sign], Act.Exp,
                        scale=2.0 / BETA, bias=NLA[0:O])
                    nc.scalar.activation(
                        Yhi[:, sign], Lhi[:, sign], Act.Exp,
                        scale=2.0 / BETA, bias=NLA[0:O])

                # combine (y11-y12) - (y21-y22) + bias
                nc.vector.tensor_tensor(
                    out=D1[:], in0=Y[0:O, 0], in1=Yhi[:, 0], op=Alu.subtract)
                nc.vector.tensor_tensor(
                    out=D2[:], in0=Y[0:O, 1], in1=Yhi[:, 1], op=Alu.subtract)
                nc.vector.tensor_scalar(
                    out=D2[:], in0=D2[:], scalar1=Bt[:], scalar2=None,
                    op0=Alu.subtract)
                nc.vector.tensor_tensor(
                    out=Rt[:], in0=D1[:], in1=D2[:], op=Alu.subtract)

            if dump == "SQ":
                SQ1 = cpool.tile([NG, 2, HO * WO], f32)
                nc.scalar.activation(
                    SQ1[:, 0], SM[:, 0].rearrange("p a f -> p (a f)"), Act.Sqrt)
                nc.scalar.activation(SQ1[:, 1], SQ1[:, 0], Act.Sqrt)
                nc.sync.dma_start(y[:], SQ1[0:O, 1, :])
            elif dump == "SM0":
                nc.gpsimd.dma_start(
                    y[:].rearrange("p (a f) -> p a f", a=2),
                    SM[0:O, 0])
            elif dump == "L0":
                nc.sync.dma_start(y[:], L[0:O, 0, :])
            elif dump == "L1":
                nc.sync.dma_start(y[:], L[O:NG, 0, :])
            elif dump == "Y0":
                nc.sync.dma_start(y[:], Y[0:O, 0, :])
            elif dump == "R128":
                nc.gpsimd.dma_start(y[:].rearrange("p (a f) -> (p a) f", a=2)[0:2 * C],
                                    R128[0:2 * C, 0:HO * WO // 2])
            elif dump == "U":
                nc.sync.dma_start(y[:].rearrange("p (a f) -> (p a) f", a=2)[0:2 * C],
                                  U[0:2 * C, 0:HO * WO // 2])
            else:
                nc.sync.dma_start(y[:], Rt[:])

    nc.compile()
    _CACHE[key] = nc
    return nc


def _get_runner(reps=1, outer=1, dump=None):
    """Cached jitted SPMD executor (replicates bass2jax.run_bass_via_pjrt but
    reuses the jitted callable across calls so we don't re-trace every time)."""
    key = ("run", reps, outer, dump)
    if key in _CACHE:
        return _CACHE[key]

    import jax
    from jax.sharding import Mesh, PartitionSpec
    try:
        from jax.experimental.shard_map import shard_map
    except ImportError:  # newer jax
        from jax.shard_map import shard_map
    from concourse import bass2jax, mybir

    nc = _build_program(reps, outer, dump)
    bass2jax.install_neuronx_cc_hook()

    partition_name = nc.partition_id_tensor.name if nc.partition_id_tensor else None
    in_names, out_names, out_avals, zero_outs = [], [], [], []
    for alloc in nc.m.functions[0].allocations:
        if not isinstance(alloc, mybir.MemoryLocationSet):
            continue
        name = alloc.memorylocations[0].name
        if alloc.kind == "ExternalInput":
            if name != partition_name:
                in_names.append(name)
        elif alloc.kind == "ExternalOutput":
            shape = tuple(alloc.tensor_shape)
            dtype = mybir.dt.np(alloc.dtype)
            out_names.append(name)
            out_avals.append(jax.core.ShapedArray(shape, dtype))
            zero_outs.append(np.zeros(shape, dtype))
    n_params = len(in_names)
    n_outs = len(out_avals)
    all_in_names = list(in_names) + list(out_names)
    if partition_name is not None:
        all_in_names.append(partition_name)
    donate = tuple(range(n_params, n_params + n_outs))

    def _body(*args):
        operands = list(args)
        if partition_name is not None:
            operands.append(bass2jax.partition_id_tensor())
        outs = bass2jax._bass_exec_p.bind(
            *operands,
            out_avals=tuple(out_avals),
            in_names=tuple(all_in_names),
            out_names=tuple(out_names),
            lowering_input_output_aliases=(),
            sim_require_finite=True,
            sim_require_nnan=True,
            nc=nc,
        )
        return tuple(outs)

    devices = jax.devices()[:NCORES]
    mesh = Mesh(np.asarray(devices), ("core",))
    sharded = jax.jit(
        shard_map(_body, mesh=mesh,
                  in_specs=(PartitionSpec("core"),) * (n_params + n_outs),
                  out_specs=(PartitionSpec("core"),) * n_outs,
                  check_rep=False),
        donate_argnums=donate,
        keep_unused=True,
    )

    def run(in_maps):
        concat_in = [
            np.concatenate([np.asarray(m[name]) for m in in_maps], axis=0)
            for name in in_names
        ]
        concat_zeros = [
            np.zeros((NCORES * z.shape[0], *z.shape[1:]), z.dtype)
            for z in zero_outs
        ]
        out_arrs = sharded(*concat_in, *concat_zeros)
        return [
            {name: np.asarray(out_arrs[i]).reshape(NCORES, *out_avals[i].shape)[c]
             for i, name in enumerate(out_names)}
            for c in range(NCORES)
        ]

    _CACHE[key] = run
    return run


def _make_in_maps(x, k1, k2, bias):
    # host-side layout prep (sharding + packing + transpose only)
    # kT[c, ij*128 + e*64 + o] = k_e[i, j, c, o]
    kk = np.stack([k1, k2], axis=3)           # [fh, fw, c, e, o]
    kT = np.ascontiguousarray(
        kk.transpose(2, 0, 1, 3, 4).reshape(C, FH * FW * NG)).astype(np.float32)
    biasb = np.ascontiguousarray(bias.reshape(O, 1).astype(np.float32))
    in_maps = []
    for b in range(NCORES):
        xb = x[b].reshape(C, H * W)
        xp = np.concatenate([xb, -xb], axis=0).astype(np.float32)
        in_maps.append({"xp": np.ascontiguousarray(xp), "kT": kT,
                        "biasb": biasb})
    return in_maps


def kernel(x, k1, k2, bias, reps=1, outer=1, dump=None):
    x = np.asarray(x, dtype=np.float32)
    k1 = np.asarray(k1, dtype=np.float32)
    k2 = np.asarray(k2, dtype=np.float32)
    bias = np.asarray(bias, dtype=np.float32)

    run = _get_runner(reps, outer, dump)
    results = run(_make_in_maps(x, k1, k2, bias))
    out = np.empty((B, O, HO, WO), dtype=np.float32)
    for b in range(NCORES):
        out[b] = results[b]["y"].reshape(O, HO, WO)
    return out


# revision 30
# speedup vs baseline: 1.0318x; 1.0318x over previous
"""Bass/Trainium2 kernel for nn_BipolarMorphological2D.

Math: reference computes, per branch,
    y = exp(max_p(log(max(patch, 0.1)) + k[p, o]))
      = max_p(max(patch, 0.1) * exp(k)[p, o])
a tropical (max-times) matmul with positive operands.  We approximate the
max-reduction with a high-order p-norm computed on the TensorEngine:
    max_p(z_p) ~= (sum_p z_p^beta)^(1/beta),  beta = 112
Since (u * e)^beta = u^beta * e^beta, each S = sum_p u^beta E^beta is a plain
matmul of patch powers against exp(beta*k) -- PE work instead of DVE work.
The p-norm overestimates when several z_p tie near the max; to suppress that,
P=288 is split into 5 chunks (pairs of 3x3 tap positions), each chunk gets its
own p-norm, and the DVE takes the exact max across chunk sums (monotone, so the
max can be taken on raw S values before the 1/beta root).  Measured rel-l2 vs
the exact reference on hardware: 9.96e-3 (threshold 2e-2).

Scaling: alpha=0.28 keeps S = sum (alpha*u*E)^112 in [e^-79, e^44]; one ACT
Sqrt pass then fits it into Ln's hardware window (2^-64, 2^64).

Per core (one image, batch-parallel across 8 cores):
  DVE:  clamp+scale u = (x max 0.1)*alpha; u^112 = u^16*u^32*u^64 products;
        20 chunk-max folds; final combine.  ACT: squaring chain to u^64 (fp32
        internal), exp(112 k), sqrt+ln+exp root.  PE: 36 matmuls, K=32, N=450
        (h,w)-strided AP covering only valid output pixels, ij-pairs
        accumulated in PSUM.  The squaring chain is split into two halo'd
        column halves so matmuls overlap it; sign 0's root overlaps sign 1's
        matmuls; the e2->e1 partition move rides a small Lhi DMA overlapped
        with the exp stage.
"""

import numpy as np

B, C, H, W, O = 8, 32, 32, 32, 64
FH, FW = 3, 3
HO, WO = H - FH + 1, W - FW + 1  # 30, 30
NG = 2 * O                       # 128 output groups: (kernel e1/e2) x (o)
NCORES = 8

BETA = 112
ALPHA = 0.28
SHIFT = 0.1
NSQ = 6                          # squarings to u^64; u^112 = u^64*u^32*u^16
HHALF = HO // 2                  # 15 rows per matmul half
NFREE = HHALF * WO               # 450 columns per matmul (one PSUM bank)

# chunk structure: 5 chunks = pairs of ij taps (last chunk is ij=8 alone)
CHUNKS = [(0, 1), (2, 3), (4, 5), (6, 7), (8,)]

_CACHE = {}


def _build_program(reps=1, outer=1, dump=None):
    key = ("nc", reps, outer, dump)
    if key in _CACHE:
        return _CACHE[key]

    import concourse.mybir as mybir
    import concourse.tile as tile
    from concourse import bacc

    f32 = mybir.dt.float32
    bf16 = mybir.dt.bfloat16
    Alu = mybir.AluOpType
    Act = mybir.ActivationFunctionType

    nc = bacc.Bacc()

    xp = nc.dram_tensor("xp", [2 * C, H * W], f32, kind="ExternalInput")
    kT = nc.dram_tensor("kT", [C, FH * FW * NG], f32, kind="ExternalInput")
    biasb = nc.dram_tensor("biasb", [O, 1], f32, kind="ExternalInput")
    y = nc.dram_tensor("y", [O, HO * WO], f32, kind="ExternalOutput")

    with tile.TileContext(nc) as tc:
        with tc.tile_pool(name="const", bufs=1) as cpool, \
             tc.tile_pool(name="psum", bufs=8, space="PSUM") as ppool:

            HWP = 17 * W
            KT = cpool.tile([C, FH * FW, NG], f32)
            nc.sync.dma_start(KT[:], kT[:].rearrange("c (a g) -> c a g", g=NG))
            XP = [cpool.tile([2 * C, HWP], f32, name=f'x{p}') for p in range(2)]
            nc.sync.dma_start(XP[0][:], xp[:, 0:HWP])
            nc.sync.dma_start(XP[1][:], xp[:, HHALF * W:])
            Bt = cpool.tile([O, 1], f32)
            nc.sync.dma_start(Bt[:], biasb[:])
            NLA = cpool.tile([NG, 1], f32)
            nc.gpsimd.memset(NLA[:], -float(np.log(ALPHA)))

            # exp(beta * k) in bf16 on rows 0:32, then DMA-duplicate to rows
            # 32:64 (matmul lhsT must share base partition with each sign's rhs)
            ET = cpool.tile([2 * C, FH * FW, NG], bf16)
            nc.scalar.activation(ET[0:C], KT[:], Act.Exp, scale=float(BETA))
            nc.sync.dma_start(ET[C:2 * C], ET[0:C])

            UP = [cpool.tile([2 * C, HWP], f32, name=f'u{p}') for p in range(2)]
            Upow = [cpool.tile([2 * C, HWP], f32, name=f'upow{n}')
                    for n in range(2 * NSQ)]
            M1P = [cpool.tile([2 * C, HWP], f32, name=f'm1{p}') for p in range(2)]
            RP = [cpool.tile([2 * C, HWP], bf16, name=f'r{p}') for p in range(2)]
            SM = cpool.tile([NG, 2, 2, NFREE], bf16)   # (sign, half) chunk-max
            Q1 = cpool.tile([NG, 2, HO * WO], f32)
            Q2 = cpool.tile([NG, 2, HO * WO], f32)
            L = cpool.tile([NG, 2, HO * WO], f32)
            Y = cpool.tile([NG, 2, HO * WO], f32)
            Lhi = cpool.tile([O, 2, HO * WO], f32)
            Yhi = cpool.tile([O, 2, HO * WO], f32)
            D1 = cpool.tile([O, HO * WO], f32)
            D2 = cpool.tile([O, HO * WO], f32)
            Rt = cpool.tile([O, HO * WO], f32)

            for _ in range(reps):
                # u = (x max 0.1) * alpha, both signs packed on 64
                # partitions; the squaring chain runs per column part (rows
                # 0:17 and 15:32 of the image, 2-row halo) so part A's
                # matmuls can start while part B is still powering.
                # part 0's squaring chain on ACT, part 1's on DVE -- the two
                # engines power their halves concurrently
                for p in range(2):
                    nc.vector.tensor_scalar(
                        out=UP[p][:], in0=XP[p][:],
                        scalar1=SHIFT, scalar2=ALPHA,
                        op0=Alu.max, op1=Alu.mult)
                for p in range(2):
                    prev = UP[p]
                    for sq in range(NSQ):
                        o = Upow[p * NSQ + sq]
                        if p == 0:
                            nc.scalar.activation(o[:], prev[:], Act.Square)
                        else:
                            nc.vector.tensor_tensor(
                                out=o[:], in0=prev[:], in1=prev[:],
                                op=Alu.mult)
                        prev = o
                    nc.vector.tensor_tensor(
                        out=M1P[p][:], in0=Upow[p * NSQ + 3][:],
                        in1=Upow[p * NSQ + 4][:], op=Alu.mult)
                    nc.vector.tensor_tensor(
                        out=RP[p][:], in0=M1P[p][:], in1=Upow[p * NSQ + 5][:],
                        op=Alu.mult)

                Rvp = [RP[p][:].rearrange("p (h w) -> p h w", w=W)
                       for p in range(2)]

                # tropical matmuls + chunk-max folds, sign-outer so sign 0's
                # root phase overlaps sign 1's matmul phase.  Within a chunk
                # the two h-halves share one LDWEIGHTS per tap.
                for sign in range(2):
                    for ci, chunk in enumerate(CHUNKS):
                        pss = []
                        for half in range(2):
                            ps = ppool.tile([NG, NFREE], f32, name="ps")
                            pss.append(ps)
                        for m, ij in enumerate(chunk):
                            i, j = divmod(ij, FW)
                            for half in range(2):
                                rhs = Rvp[half][C * sign:C * (sign + 1),
                                               i:i + HHALF, j:j + WO]
                                nc.tensor.matmul(
                                    pss[half][:],
                                    ET[C * sign:C * (sign + 1), ij, :], rhs,
                                    start=(m == 0), stop=(m == len(chunk) - 1))
                        for half in range(2):
                            dstv = SM[:, sign, half, :]
                            if ci == 0:
                                nc.vector.tensor_scalar(
                                    out=dstv, in0=pss[half][:], scalar1=0.0,
                                    scalar2=None, op0=Alu.max)
                            else:
                                nc.vector.tensor_tensor(
                                    out=dstv, in0=dstv, in1=pss[half][:],
                                    op=Alu.max)
                    # one Sqrt pass compresses S into Ln's (2^-64, 2^64)
                    # window (S in [e^-79, e^44] -> [e^-40, e^22]); sign 0's
                    # runs under sign 1's matmul phase
                    nc.scalar.activation(
                        Q1[:, sign], SM[:, sign].rearrange("p a f -> p (a f)"),
                        Act.Sqrt)

                # root tail: jumbo Ln across signs (keeps the scheduler from
                # thrashing ACT table sets), early Lhi DMA, then per-block exps
                nc.scalar.activation(
                    L[:].rearrange("p a f -> p (a f)"),
                    Q1[:].rearrange("p a f -> p (a f)"), Act.Ln)
                nc.sync.dma_start(Lhi[:], L[O:NG])
                for sign in range(2):
                    nc.scalar.activation(
                        Y[0:O, sign], L[0:O, sign], Act.Exp,
                        scale=2.0 / BETA, bias=NLA[0:O])
                    nc.scalar.activation(
                        Yhi[:, sign], Lhi[:, sign], Act.Exp,
                        scale=2.0 / BETA, bias=NLA[0:O])

                # combine (y11-y12) - (y21-y22) + bias
                nc.vector.tensor_tensor(
                    out=D1[:], in0=Y[0:O, 0], in1=Yhi[:, 0], op=Alu.subtract)
                nc.vector.tensor_tensor(
                    out=D2[:], in0=Y[0:O, 1], in1=Yhi[:, 1], op=Alu.subtract)
                nc.vector.tensor_scalar(
                    out=D2[:], in0=D2[:], scalar1=Bt[:], scalar2=None,
                    op0=Alu.subtract)
                nc.vector.tensor_tensor(
                    out=Rt[:], in0=D1[:], in1=D2[:], op=Alu.subtract)

            if dump == "SQ":
                SQ1 = cpool.tile([NG, 2, HO * WO], f32)
                nc.scalar.activation(
                    SQ1[:, 0], SM[:, 0].rearrange("p a f -> p (a f)"), Act.Sqrt)
                nc.scalar.activation(SQ1[:, 1], SQ1[:, 0], Act.Sqrt)
                nc.sync.dma_start(y[:], SQ1[0:O, 1, :])
            elif dump == "SM0":
                nc.gpsimd.dma_start(
                    y[:].rearrange("p (a f) -> p a f", a=2),
                    SM[0:O, 0])
            elif dump == "L0":
                nc.sync.dma_start(y[:], L[0:O, 0, :])
            elif dump == "L1":
                nc.sync.dma_start(y[:], L[O:NG, 0, :])
            elif dump == "Y0":
                nc.sync.dma_start(y[:], Y[0:O, 0, :])
            elif dump == "R128":
                nc.gpsimd.dma_start(y[:].rearrange("p (a f) -> (p a) f", a=2)[0:2 * C],
                                    R128[0:2 * C, 0:HO * WO // 2])
            elif dump == "U":
                nc.sync.dma_start(y[:].rearrange("p (a f) -> (p a) f", a=2)[0:2 * C],
                                  U[0:2 * C, 0:HO * WO // 2])
            else:
                nc.sync.dma_start(y[:], Rt[:])

    nc.compile()
    _CACHE[key] = nc
    return nc


def _get_runner(reps=1, outer=1, dump=None):
    """Cached jitted SPMD executor (replicates bass2jax.run_bass_via_pjrt but
    reuses the jitted callable across calls so we don't re-trace every time)."""
    key = ("run", reps, outer, dump)
    if key in _CACHE:
        return _CACHE[key]

    import jax
    from jax.sharding import Mesh, PartitionSpec
    try:
        from jax.experimental.shard_map import shard_map
    except ImportError:  # newer jax
        from jax.shard_map import shard_map
    from concourse import bass2jax, mybir

    nc = _build_program(reps, outer, dump)
    bass2jax.install_neuronx_cc_hook()

    partition_name = nc.partition_id_tensor.name if nc.partition_id_tensor else None
    in_names, out_names, out_avals, zero_outs = [], [], [], []
    for alloc in nc.m.functions[0].allocations:
        if not isinstance(alloc, mybir.MemoryLocationSet):
            continue
        name = alloc.memorylocations[0].name
        if alloc.kind == "ExternalInput":
            if name != partition_name:
                in_names.append(name)
        elif alloc.kind == "ExternalOutput":
            shape = tuple(alloc.tensor_shape)
            dtype = mybir.dt.np(alloc.dtype)
            out_names.append(name)
            out_avals.append(jax.core.ShapedArray(shape, dtype))
            zero_outs.append(np.zeros(shape, dtype))
    n_params = len(in_names)
    n_outs = len(out_avals)
    all_in_names = list(in_names) + list(out_names)
    if partition_name is not None:
        all_in_names.append(partition_name)
    donate = tuple(range(n_params, n_params + n_outs))

    def _body(*args):
        operands = list(args)
        if partition_name is not None:
            operands.append(bass2jax.partition_id_tensor())
        outs = bass2jax._bass_exec_p.bind(
            *operands,
            out_avals=tuple(out_avals),
            in_names=tuple(all_in_names),
            out_names=tuple(out_names),
            lowering_input_output_aliases=(),
            sim_require_finite=True,
            sim_require_nnan=True,
            nc=nc,
        )
        return tuple(outs)

    devices = jax.devices()[:NCORES]
    mesh = Mesh(np.asarray(devices), ("core",))
    sharded = jax.jit(
        shard_map(_body, mesh=mesh,
                  in_specs=(PartitionSpec("core"),) * (n_params + n_outs),
                  out_specs=(PartitionSpec("core"),) * n_outs,
                  check_rep=False),
        donate_argnums=donate,
        keep_unused=True,
    )

    def run(in_maps):
        concat_in = [
            np.concatenate([np.asarray(m[name]) for m in in_maps], axis=0)
            for name in in_names
        ]
        concat_zeros = [
            np.zeros((NCORES * z.shape[0], *z.shape[1:]), z.dtype)
            for z in zero_outs
        ]
        out_arrs = sharded(*concat_in, *concat_zeros)
        return [
            {name: np.asarray(out_arrs[i]).reshape(NCORES, *out_avals[i].shape)[c]
             for i, name in enumerate(out_names)}
            for c in range(NCORES)
        ]

    _CACHE[key] = run
    return run


def _make_in_maps(x, k1, k2, bias):
    # host-side layout prep (sharding + packing + transpose only)
    # kT[c, ij*128 + e*64 + o] = k_e[i, j, c, o]
    kk = np.stack([k1, k2], axis=3)           # [fh, fw, c, e, o]
    kT = np.ascontiguousarray(
        kk.transpose(2, 0, 1, 3, 4).reshape(C, FH * FW * NG)).astype(np.float32)
    biasb = np.ascontiguousarray(bias.reshape(O, 1).astype(np.float32))
    in_maps = []
    for b in range(NCORES):
        xb = x[b].reshape(C, H * W)
        xp = np.concatenate([xb, -xb], axis=0).astype(np.float32)
        in_maps.append({"xp": np.ascontiguousarray(xp), "kT": kT,
                        "biasb": biasb})
    return in_maps


def kernel(x, k1, k2, bias, reps=1, outer=1, dump=None):
    x = np.asarray(x, dtype=np.float32)
    k1 = np.asarray(k1, dtype=np.float32)
    k2 = np.asarray(k2, dtype=np.float32)
    bias = np.asarray(bias, dtype=np.float32)

    run = _get_runner(reps, outer, dump)
    results = run(_make_in_maps(x, k1, k2, bias))
    out = np.empty((B, O, HO, WO), dtype=np.float32)
    for b in range(NCORES):
        out[b] = results[b]["y"].reshape(O, HO, WO)
    return out
